# revision 18
# baseline (speedup 1.0000x reference)
"""Trainium2 Bass kernel for the mixture log-likelihood loss.

reference:
    log_otu = log(otu_dist + EPS)                       # (K=64, O=1024)
    lse[n,k] = counts[n] . log_otu[k] + log(comm+EPS)[k]
    out = sum_n logsumexp_k(lse[n, :])

Strategy (8 NeuronCores, data-parallel over N):
  * counts rows are small integers -> exact in fp8 e4m3. Cast on host,
    quartering HBM traffic (the kernel is memory-bound on counts). Falls
    back to an identically-structured bf16 module if the data ever stops
    being fp8-exact.
  * log_otu is quantized to a single fp8 plane (measured end-to-end error
    2.1e-3 against the f32 reference, an order under the 2e-2 gate); fp8
    matmuls run in DoubleRow perf mode (256-deep contraction per pass),
    pairing adjacent 128-wide O-chunks of counts (stationary) against the
    matching log_otu chunk-pair (moving).
  * The mixture prior never touches the matmul path: logsumexp is
    computed as max + ln(sum_k w_k * exp(raw_k - max)) with
    w_k = comm_k + EPS, so the per-block fused tensor_tensor_reduce
    (multiply by w, accumulate) replaces both the prior add and the
    separate sum-exp reduction.
  * All counts DMAs are issued up front (the full 12.8 MB shard fits in
    SBUF), so the 16 DMA engines stream back-to-back with no compute
    back-pressure; compute chases the stream.
  * Eight 128-particle blocks share one PSUM bank (128, 8, 64);
    reduce_max(negate=True) feeds exp's per-partition bias.
  * All Ln work is deferred to end-of-kernel activations over the
    (128, 98) gathered sums (avoids exp/ln ACT-table ping-pong).
  * Per-core partial sum is reduced over partitions with a tiny f32
    matmul against ones; the host adds the 8 scalars and analytically
    removes the zero-row padding contribution.
"""

import numpy as np
import ml_dtypes

N, K, O = 100000, 64, 1024
EPS = 1e-6
CORES = 8
NSHARD = N // CORES          # 12500
BLK = 128
NBLK = 98                    # ceil(12500 / 128)
NPAD = NBLK * BLK            # 12544
BPS = 14                     # blocks per superblock (even: pairs don't split)
SBS = NBLK // BPS            # 7 superblocks
PAD_ROWS = NPAD - NSHARD     # 44 zero rows per core
GRP = 8                      # blocks per PSUM group (one full PSUM bank)

_cache = {}


def _build_module(use_fp8):
    import concourse.bacc as bacc
    import concourse.tile as tile
    from concourse import mybir

    # Force all activations (Exp/Ln/Copy) onto the one ACT table set that
    # contains them all — otherwise every Exp<->Ln switch pays a ~1.3us
    # ACT_TABLE_LOAD. Other sets are blanked (positions kept so the
    # act_func_set_id -> act_info.json index mapping stays valid).
    if not getattr(bacc, "_act_tables_patched", False):
        _orig_get = bacc.get_activation_tables

        def _only_ln_exp(arch):
            tabs = _orig_get(arch)
            return {
                name: (fns if name == "natural_log_exp_and_others" else set())
                for name, fns in tabs.items()
            }

        bacc.get_activation_tables = _only_ln_exp
        bacc._act_tables_patched = True

    f32 = mybir.dt.float32
    bf16 = mybir.dt.bfloat16
    cdt = mybir.dt.float8e4 if use_fp8 else bf16
    AX = mybir.AxisListType.X
    AF = mybir.ActivationFunctionType
    ALU = mybir.AluOpType
    DR = mybir.MatmulPerfMode.DoubleRow

    nc = bacc.Bacc("TRN2", target_bir_lowering=False, debug=False,
                   num_devices=CORES)
    cnts = nc.dram_tensor("cnts", [SBS, 128, BPS * 8, BLK], cdt,
                          kind="ExternalInput").ap()
    hiw = nc.dram_tensor("hiw", [128, 8, K], cdt,
                         kind="ExternalInput").ap()
    priorb = nc.dram_tensor("priorb", [1, GRP, K], bf16,
                            kind="ExternalInput").ap()
    onesb = nc.dram_tensor("onesb", [1, 128], bf16,
                           kind="ExternalInput").ap()
    ones = nc.dram_tensor("ones", [128, 1], f32, kind="ExternalInput").ap()
    out = nc.dram_tensor("out", [1, 1], f32, kind="ExternalOutput").ap()

    with tile.TileContext(nc, num_cores=CORES) as tc:
        with (
            tc.tile_pool(name="const", bufs=1) as const,
            tc.tile_pool(name="cnt", bufs=SBS) as cnt_pool,
            tc.tile_pool(name="work", bufs=4) as work,
            tc.tile_pool(name="psum", bufs=6, space="PSUM") as psum_pool,
            tc.tile_pool(name="fpsum", bufs=1, space="PSUM") as fpsum_pool,
        ):
            # constants ride the SWDGE queue so the big counts DMAs own HWDGE
            hi_sb = const.tile([128, 8, K], cdt)
            nc.gpsimd.dma_start(out=hi_sb[:], in_=hiw)
            priorb_sb = const.tile([1, GRP, K], bf16)
            nc.gpsimd.dma_start(out=priorb_sb[:], in_=priorb)
            onesb_sb = const.tile([1, 128], bf16)
            nc.gpsimd.dma_start(out=onesb_sb[:], in_=onesb)
            ones_sb = const.tile([128, 1], f32)
            nc.gpsimd.dma_start(out=ones_sb[:], in_=ones)
            mg_all = const.tile([128, NBLK], f32)
            sg_all = const.tile([128, NBLK], bf16)
            # touch Exp and Ln once (into a slice that is later fully
            # overwritten, so DCE keeps it) so both ACT table loads overlap
            # the DMA-bound head instead of landing in the kernel tail
            warm = const.tile([1, 1], f32)
            nc.vector.memset(warm[:], 1.0)
            nc.scalar.activation(mg_all[0:1, 0:1], warm[:], AF.Exp)
            nc.scalar.activation(mg_all[0:1, 0:1], warm[:], AF.Ln)

            # Issue ALL counts DMAs up front: the full shard fits in SBUF,
            # so the DMA engines never wait on compute. Fine-grained splits
            # at the head (compute starts early) and tail (short drain).
            # Descriptor generation (~0.7us per dma_start) is spread across
            # four engines' queues so the head is not serialized on sync.
            # All issues ride the sync engine: it is otherwise idle, and
            # issuing from scalar would queue descriptors behind the Exps,
            # starving the DMA engines mid-kernel.
            cnt_tiles = []
            for s in range(SBS):
                cnt = cnt_pool.tile([128, BPS * 8, BLK], cdt)
                if s == 0:
                    splits = [0, 1, 2, 4, 7, 10, BPS]
                elif s == SBS - 1:
                    splits = [0, 7, 11, BPS]
                else:
                    splits = [0, 7, BPS]
                for a, b in zip(splits, splits[1:]):
                    nc.sync.dma_start(out=cnt[:, a * 8:b * 8, :],
                                      in_=cnts[s, :, a * 8:b * 8, :])
                cnt_tiles.append(cnt)

            def block_ap(b):
                return cnt_tiles[b // BPS], (b % BPS) * 8

            NG = (NBLK - 2) // GRP
            groups = [(GRP * q, GRP) for q in range(NG)]
            groups.append((GRP * NG, 2))
            # incremental logsumexp tail: ln/sub/sum as columns complete
            segs = [(0, 24), (24, 48), (48, 72), (72, 96), (96, NBLK)]
            ls = const.tile([128, NBLK], f32)
            t3 = const.tile([128, NBLK], f32)
            acc8 = const.tile([128, len(segs)], f32)
            seg_after = {b: i for i, (a, b) in enumerate(segs)}

            def emit_seg(i):
                a, b = segs[i]
                nc.scalar.activation(ls[:, a:b], sg_all[:, a:b], AF.Ln)
                nc.vector.tensor_sub(t3[:, a:b], ls[:, a:b], mg_all[:, a:b])
                nc.vector.reduce_sum(acc8[:, i:i + 1], t3[:, a:b], axis=AX)

            from concourse.bass import AP as BassAP

            def k_broadcast(ap_2d):
                # view a [128, gn] slice as [128, gn, K] with 0-stride K dim
                return BassAP(ap_2d.tensor, ap_2d.offset,
                              list(ap_2d.ap) + [[0, K]])

            for g0, gn in groups:
                B8 = psum_pool.tile([128, GRP, K], mybir.dt.float32)
                # prior lands first via a 1-row bf16 matmul: zero-initializes
                # the group's PSUM bank and adds log(comm+EPS) to every block
                nc.tensor.matmul(
                    B8[:, :gn, :],
                    lhsT=onesb_sb[:],
                    rhs=priorb_sb[:, :gn, :],
                    start=True, stop=False,
                    skip_group_check=True,
                )
                for j in range(gn):                 # block within group
                    tile_b, off = block_ap(g0 + j)
                    # plain fp8 matmuls: DoubleRow halves stream cycles but
                    # the PE power-cap halves the clock in exchange, while
                    # the (serialized, unhidden) ldweights double in time —
                    # measured net loss. Non-DR runs at full clock.
                    for c in range(8):
                        nc.tensor.matmul(
                            B8[:, j, :],
                            lhsT=tile_b[:, off + c, :],
                            rhs=hi_sb[:, c, :],
                            start=False,
                            stop=(j == gn - 1 and c == 7),
                            skip_group_check=True,
                        )
                nc.vector.reduce_max(mg_all[:, g0:g0 + gn], B8[:, :gn, :],
                                     axis=AX, negate=True)
                # x - max via one broadcast add (mg holds the negated max);
                # bf16 out feeds one grouped Exp on the scalar engine
                t5 = work.tile([128, GRP, K], bf16, tag="t5")
                nc.vector.tensor_add(t5[:, :gn, :], B8[:, :gn, :],
                                     k_broadcast(mg_all[:, g0:g0 + gn]))
                e8 = work.tile([128, GRP, K], bf16, tag="e8")
                nc.scalar.activation(e8[:, :gn, :], t5[:, :gn, :], AF.Exp)
                with nc.allow_low_precision("sum of <=64 max-normalized "
                                            "exps; bf16 keeps 2x DVE mode"):
                    nc.vector.reduce_sum(sg_all[:, g0:g0 + gn],
                                         e8[:, :gn, :], axis=AX)
                if g0 + gn in seg_after:
                    emit_seg(seg_after[g0 + gn])

            accp = const.tile([128, 1], f32)
            nc.vector.reduce_sum(accp[:], acc8[:], axis=AX)
            fin_ps = fpsum_pool.tile([1, 1], f32)
            nc.tensor.matmul(fin_ps[:], lhsT=accp[:], rhs=ones_sb[:],
                             start=True, stop=True)
            fin_sb = const.tile([1, 1], f32)
            nc.scalar.copy(fin_sb[:], fin_ps[:])
            nc.sync.dma_start(out=out, in_=fin_sb[:])

    nc.finalize()
    return nc


def _prep_inputs(counts, otu_dist, comm_dist, use_fp8):
    np_dt = ml_dtypes.float8_e4m3 if use_fp8 else ml_dtypes.bfloat16
    log_otu = np.log(otu_dist.astype(np.float32) + np.float32(EPS))
    hi = log_otu.astype(np_dt)
    # [p, c, k] = x[k, c*128 + p]
    hiw = np.ascontiguousarray(
        hi.reshape(K, 8, BLK).transpose(2, 1, 0))      # (128, 8, 64)

    prior_vec = np.log(comm_dist.astype(np.float32) + np.float32(EPS))
    prior_b16 = prior_vec.astype(ml_dtypes.bfloat16)
    priorb = np.ascontiguousarray(
        np.broadcast_to(prior_b16[None, None, :], (1, GRP, K)))
    onesb = np.ones((1, 128), ml_dtypes.bfloat16)
    ones = np.ones((128, 1), np.float32)

    counts_q = counts.astype(np_dt)
    shards = []
    for i in range(CORES):
        sh = counts_q[i * NSHARD:(i + 1) * NSHARD]
        shp = np.zeros((NPAD, O), np_dt)
        shp[:NSHARD] = sh
        # (s, b, j, c, p) -> (s, p, b, c, j)
        arr = shp.reshape(SBS, BPS, BLK, 8, BLK).transpose(0, 4, 1, 3, 2)
        shards.append(np.ascontiguousarray(arr).reshape(SBS, 128,
                                                        BPS * 8, BLK))

    in_maps = [
        {"cnts": shards[i], "hiw": hiw, "priorb": priorb, "onesb": onesb,
         "ones": ones}
        for i in range(CORES)
    ]
    # per-particle value contributed by each all-zero padding row:
    # raw = bf16(prior)  ->  summand = logsumexp(bf16(prior))
    pad_val = _np_logsumexp(prior_b16.astype(np.float64))
    return in_maps, pad_val


def _np_logsumexp(v):
    m = np.max(v)
    return m + np.log(np.sum(np.exp(v - m)))


def kernel(counts, otu_dist, comm_dist):
    from concourse.bass_utils import run_bass_kernel_spmd

    counts = np.asarray(counts)
    fp8 = ml_dtypes.float8_e4m3
    use_fp8 = bool(
        np.array_equal(counts.astype(fp8).astype(np.float32),
                       counts.astype(np.float32)))

    key = ("nc", use_fp8)
    if key not in _cache:
        _cache[key] = _build_module(use_fp8)
    nc = _cache[key]

    in_maps, pad_val = _prep_inputs(counts, np.asarray(otu_dist),
                                    np.asarray(comm_dist), use_fp8)
    res = run_bass_kernel_spmd(nc, in_maps, list(range(CORES)))
    total = sum(float(res.results[c]["out"][0, 0]) for c in range(CORES))
    total -= CORES * PAD_ROWS * pad_val
    return np.float32(total)


# revision 21
# speedup vs baseline: 1.0091x; 1.0091x over previous
"""Trainium2 Bass kernel for the mixture log-likelihood loss.

reference:
    log_otu = log(otu_dist + EPS)                       # (K=64, O=1024)
    lse[n,k] = counts[n] . log_otu[k] + log(comm+EPS)[k]
    out = sum_n logsumexp_k(lse[n, :])

Strategy (8 NeuronCores, data-parallel over N):
  * counts rows are small integers -> exact in fp8 e4m3. Cast on host,
    quartering HBM traffic (the kernel is memory-bound on counts). Falls
    back to an identically-structured bf16 module if the data ever stops
    being fp8-exact.
  * log_otu is quantized to a single fp8 plane (measured end-to-end error
    2.1e-3 against the f32 reference, an order under the 2e-2 gate); fp8
    matmuls run in DoubleRow perf mode (256-deep contraction per pass),
    pairing adjacent 128-wide O-chunks of counts (stationary) against the
    matching log_otu chunk-pair (moving).
  * The mixture prior never touches the matmul path: logsumexp is
    computed as max + ln(sum_k w_k * exp(raw_k - max)) with
    w_k = comm_k + EPS, so the per-block fused tensor_tensor_reduce
    (multiply by w, accumulate) replaces both the prior add and the
    separate sum-exp reduction.
  * All counts DMAs are issued up front (the full 12.8 MB shard fits in
    SBUF), so the 16 DMA engines stream back-to-back with no compute
    back-pressure; compute chases the stream.
  * Eight 128-particle blocks share one PSUM bank (128, 8, 64);
    reduce_max(negate=True) feeds exp's per-partition bias.
  * All Ln work is deferred to end-of-kernel activations over the
    (128, 98) gathered sums (avoids exp/ln ACT-table ping-pong).
  * Per-core partial sum is reduced over partitions with a tiny f32
    matmul against ones; the host adds the 8 scalars and analytically
    removes the zero-row padding contribution.
"""

import numpy as np
import ml_dtypes

N, K, O = 100000, 64, 1024
EPS = 1e-6
CORES = 8
NSHARD = N // CORES          # 12500
BLK = 128
NBLK = 98                    # ceil(12500 / 128)
NPAD = NBLK * BLK            # 12544
BPS = 14                     # blocks per superblock (even: pairs don't split)
SBS = NBLK // BPS            # 7 superblocks
PAD_ROWS = NPAD - NSHARD     # 44 zero rows per core
GRP = 8                      # blocks per PSUM group (one full PSUM bank)

_cache = {}


def _build_module(use_fp8):
    import concourse.bacc as bacc
    import concourse.tile as tile
    from concourse import mybir

    # Force all activations (Exp/Ln/Copy) onto the one ACT table set that
    # contains them all — otherwise every Exp<->Ln switch pays a ~1.3us
    # ACT_TABLE_LOAD. Other sets are blanked (positions kept so the
    # act_func_set_id -> act_info.json index mapping stays valid).
    if not getattr(bacc, "_act_tables_patched", False):
        _orig_get = bacc.get_activation_tables

        def _only_ln_exp(arch):
            tabs = _orig_get(arch)
            return {
                name: (fns if name == "natural_log_exp_and_others" else set())
                for name, fns in tabs.items()
            }

        bacc.get_activation_tables = _only_ln_exp
        bacc._act_tables_patched = True

    f32 = mybir.dt.float32
    bf16 = mybir.dt.bfloat16
    cdt = mybir.dt.float8e4 if use_fp8 else bf16
    AX = mybir.AxisListType.X
    AF = mybir.ActivationFunctionType
    ALU = mybir.AluOpType
    DR = mybir.MatmulPerfMode.DoubleRow

    nc = bacc.Bacc("TRN2", target_bir_lowering=False, debug=False,
                   num_devices=CORES)
    cnts = nc.dram_tensor("cnts", [SBS, 128, BPS * 8, BLK], cdt,
                          kind="ExternalInput").ap()
    hiw = nc.dram_tensor("hiw", [128, 8, K], cdt,
                         kind="ExternalInput").ap()
    priorb = nc.dram_tensor("priorb", [1, GRP, K], bf16,
                            kind="ExternalInput").ap()
    onesb = nc.dram_tensor("onesb", [1, 128], bf16,
                           kind="ExternalInput").ap()
    ones = nc.dram_tensor("ones", [128, 1], f32, kind="ExternalInput").ap()
    out = nc.dram_tensor("out", [1, 1], f32, kind="ExternalOutput").ap()

    with tile.TileContext(nc, num_cores=CORES) as tc:
        with (
            tc.tile_pool(name="const", bufs=1) as const,
            tc.tile_pool(name="cnt", bufs=SBS) as cnt_pool,
            tc.tile_pool(name="work", bufs=4) as work,
            tc.tile_pool(name="psum", bufs=7, space="PSUM") as psum_pool,
            tc.tile_pool(name="fpsum", bufs=1, space="PSUM") as fpsum_pool,
        ):
            # hi gates the first matmul: it goes FIRST on the sync HWDGE
            # queue (64 KB, ~0.2us). The other constants ride the gpsimd
            # SWDGE queue and are only needed later (prior matmul is last
            # in each group).
            hi_sb = const.tile([128, 8, K], cdt)
            nc.sync.dma_start(out=hi_sb[:], in_=hiw)
            priorb_sb = const.tile([1, GRP, K], bf16)
            nc.gpsimd.dma_start(out=priorb_sb[:], in_=priorb)
            onesb_sb = const.tile([1, 128], bf16)
            nc.gpsimd.dma_start(out=onesb_sb[:], in_=onesb)
            ones_sb = const.tile([128, 1], f32)
            nc.gpsimd.dma_start(out=ones_sb[:], in_=ones)
            mg_all = const.tile([128, NBLK], f32)
            sg_all = const.tile([128, NBLK], bf16)
            # touch Exp and Ln once (into a slice that is later fully
            # overwritten, so DCE keeps it) so both ACT table loads overlap
            # the DMA-bound head instead of landing in the kernel tail
            warm = const.tile([1, 1], f32)
            nc.vector.memset(warm[:], 1.0)
            nc.scalar.activation(mg_all[0:1, 0:1], warm[:], AF.Exp)
            nc.scalar.activation(mg_all[0:1, 0:1], warm[:], AF.Ln)

            # Issue ALL counts DMAs up front: the full shard fits in SBUF,
            # so the DMA engines never wait on compute. Fine-grained splits
            # at the head (compute starts early) and tail (short drain).
            # Descriptor generation (~0.7us per dma_start) is spread across
            # four engines' queues so the head is not serialized on sync.
            # All issues ride the sync engine: it is otherwise idle, and
            # issuing from scalar would queue descriptors behind the Exps,
            # starving the DMA engines mid-kernel.
            cnt_tiles = []
            for s in range(SBS):
                cnt = cnt_pool.tile([128, BPS * 8, BLK], cdt)
                if s == 0:
                    splits = [0, 1, 2, 4, 7, 10, BPS]
                elif s == SBS - 1:
                    splits = [0, 7, 11, BPS]
                else:
                    splits = [0, 7, BPS]
                for a, b in zip(splits, splits[1:]):
                    nc.sync.dma_start(out=cnt[:, a * 8:b * 8, :],
                                      in_=cnts[s, :, a * 8:b * 8, :])
                cnt_tiles.append(cnt)

            def block_ap(b):
                return cnt_tiles[b // BPS], (b % BPS) * 8

            NG = (NBLK - 2) // GRP
            groups = [(GRP * q, GRP) for q in range(NG)]
            groups.append((GRP * NG, 2))
            # incremental logsumexp tail: ln/sub/sum as columns complete
            segs = [(0, 24), (24, 48), (48, 72), (72, 96), (96, NBLK)]
            ls = const.tile([128, NBLK], f32)
            t3 = const.tile([128, NBLK], f32)
            acc8 = const.tile([128, len(segs)], f32)
            seg_after = {b: i for i, (a, b) in enumerate(segs)}

            def emit_seg(i):
                a, b = segs[i]
                nc.scalar.activation(ls[:, a:b], sg_all[:, a:b], AF.Ln)
                nc.vector.tensor_sub(t3[:, a:b], ls[:, a:b], mg_all[:, a:b])
                nc.vector.reduce_sum(acc8[:, i:i + 1], t3[:, a:b], axis=AX)

            from concourse.bass import AP as BassAP

            def k_broadcast(ap_2d):
                # view a [128, gn] slice as [128, gn, K] with 0-stride K dim
                return BassAP(ap_2d.tensor, ap_2d.offset,
                              list(ap_2d.ap) + [[0, K]])

            for gi, (g0, gn) in enumerate(groups):
                B8 = psum_pool.tile([128, GRP, K], mybir.dt.float32)
                for j in range(gn):                 # block within group
                    tile_b, off = block_ap(g0 + j)
                    # plain fp8 matmuls: DoubleRow halves stream cycles but
                    # the PE power-cap halves the clock in exchange, while
                    # the (serialized, unhidden) ldweights double in time —
                    # measured net loss. Non-DR runs at full clock.
                    for c in range(8):
                        nc.tensor.matmul(
                            B8[:, j, :],
                            lhsT=tile_b[:, off + c, :],
                            rhs=hi_sb[:, c, :],
                            start=(j == 0 and c == 0),
                            stop=False,
                            skip_group_check=True,
                        )
                # prior lands last via a 1-row bf16 matmul so the group's
                # first counts matmul is not gated on the SWDGE constants
                nc.tensor.matmul(
                    B8[:, :gn, :],
                    lhsT=onesb_sb[:],
                    rhs=priorb_sb[:, :gn, :],
                    start=False, stop=True,
                    skip_group_check=True,
                )
                nc.vector.reduce_max(mg_all[:, g0:g0 + gn], B8[:, :gn, :],
                                     axis=AX, negate=True)
                e8 = work.tile([128, GRP, K], bf16, tag="e8")
                if gi % 2 == 0:
                    # scalar path: per-block Exp reads PSUM with the negated
                    # max as per-partition bias
                    for j in range(gn):
                        nc.scalar.activation(
                            e8[:, j, :], B8[:, j, :], AF.Exp,
                            bias=mg_all[:, g0 + j:g0 + j + 1], scale=1.0)
                else:
                    # DVE path: x - max via one broadcast add, then a single
                    # grouped Exp. Alternating paths balances the two
                    # engines' queues during the end-of-kernel drain.
                    t5 = work.tile([128, GRP, K], bf16, tag="t5")
                    nc.vector.tensor_add(t5[:, :gn, :], B8[:, :gn, :],
                                         k_broadcast(mg_all[:, g0:g0 + gn]))
                    nc.scalar.activation(e8[:, :gn, :], t5[:, :gn, :],
                                         AF.Exp)
                with nc.allow_low_precision("sum of <=64 max-normalized "
                                            "exps; bf16 keeps 2x DVE mode"):
                    nc.vector.reduce_sum(sg_all[:, g0:g0 + gn],
                                         e8[:, :gn, :], axis=AX)
                if g0 + gn in seg_after:
                    emit_seg(seg_after[g0 + gn])

            accp = const.tile([128, 1], f32)
            nc.vector.reduce_sum(accp[:], acc8[:], axis=AX)
            fin_ps = fpsum_pool.tile([1, 1], f32)
            nc.tensor.matmul(fin_ps[:], lhsT=accp[:], rhs=ones_sb[:],
                             start=True, stop=True)
            fin_sb = const.tile([1, 1], f32)
            nc.scalar.copy(fin_sb[:], fin_ps[:])
            nc.sync.dma_start(out=out, in_=fin_sb[:])

    nc.finalize()
    return nc


def _prep_inputs(counts, otu_dist, comm_dist, use_fp8):
    np_dt = ml_dtypes.float8_e4m3 if use_fp8 else ml_dtypes.bfloat16
    log_otu = np.log(otu_dist.astype(np.float32) + np.float32(EPS))
    hi = log_otu.astype(np_dt)
    # [p, c, k] = x[k, c*128 + p]
    hiw = np.ascontiguousarray(
        hi.reshape(K, 8, BLK).transpose(2, 1, 0))      # (128, 8, 64)

    prior_vec = np.log(comm_dist.astype(np.float32) + np.float32(EPS))
    prior_b16 = prior_vec.astype(ml_dtypes.bfloat16)
    priorb = np.ascontiguousarray(
        np.broadcast_to(prior_b16[None, None, :], (1, GRP, K)))
    onesb = np.ones((1, 128), ml_dtypes.bfloat16)
    ones = np.ones((128, 1), np.float32)

    counts_q = counts.astype(np_dt)
    shards = []
    for i in range(CORES):
        sh = counts_q[i * NSHARD:(i + 1) * NSHARD]
        shp = np.zeros((NPAD, O), np_dt)
        shp[:NSHARD] = sh
        # (s, b, j, c, p) -> (s, p, b, c, j)
        arr = shp.reshape(SBS, BPS, BLK, 8, BLK).transpose(0, 4, 1, 3, 2)
        shards.append(np.ascontiguousarray(arr).reshape(SBS, 128,
                                                        BPS * 8, BLK))

    in_maps = [
        {"cnts": shards[i], "hiw": hiw, "priorb": priorb, "onesb": onesb,
         "ones": ones}
        for i in range(CORES)
    ]
    # per-particle value contributed by each all-zero padding row:
    # raw = bf16(prior)  ->  summand = logsumexp(bf16(prior))
    pad_val = _np_logsumexp(prior_b16.astype(np.float64))
    return in_maps, pad_val


def _np_logsumexp(v):
    m = np.max(v)
    return m + np.log(np.sum(np.exp(v - m)))


def kernel(counts, otu_dist, comm_dist):
    from concourse.bass_utils import run_bass_kernel_spmd

    counts = np.asarray(counts)
    fp8 = ml_dtypes.float8_e4m3
    use_fp8 = bool(
        np.array_equal(counts.astype(fp8).astype(np.float32),
                       counts.astype(np.float32)))

    key = ("nc", use_fp8)
    if key not in _cache:
        _cache[key] = _build_module(use_fp8)
    nc = _cache[key]

    in_maps, pad_val = _prep_inputs(counts, np.asarray(otu_dist),
                                    np.asarray(comm_dist), use_fp8)
    res = run_bass_kernel_spmd(nc, in_maps, list(range(CORES)))
    total = sum(float(res.results[c]["out"][0, 0]) for c in range(CORES))
    total -= CORES * PAD_ROWS * pad_val
    return np.float32(total)


# revision 23
# speedup vs baseline: 1.0210x; 1.0118x over previous
"""Trainium2 Bass kernel for the mixture log-likelihood loss.

reference:
    log_otu = log(otu_dist + EPS)                       # (K=64, O=1024)
    lse[n,k] = counts[n] . log_otu[k] + log(comm+EPS)[k]
    out = sum_n logsumexp_k(lse[n, :])

Strategy (8 NeuronCores, data-parallel over N):
  * counts rows are small integers -> exact in fp8 e4m3. Cast on host,
    quartering HBM traffic (the kernel is memory-bound on counts). Falls
    back to an identically-structured bf16 module if the data ever stops
    being fp8-exact.
  * log_otu is quantized to a single fp8 plane (measured end-to-end error
    2.1e-3 against the f32 reference, an order under the 2e-2 gate); fp8
    matmuls run in DoubleRow perf mode (256-deep contraction per pass),
    pairing adjacent 128-wide O-chunks of counts (stationary) against the
    matching log_otu chunk-pair (moving).
  * The mixture prior never touches the matmul path: logsumexp is
    computed as max + ln(sum_k w_k * exp(raw_k - max)) with
    w_k = comm_k + EPS, so the per-block fused tensor_tensor_reduce
    (multiply by w, accumulate) replaces both the prior add and the
    separate sum-exp reduction.
  * All counts DMAs are issued up front (the full 12.8 MB shard fits in
    SBUF), so the 16 DMA engines stream back-to-back with no compute
    back-pressure; compute chases the stream.
  * Eight 128-particle blocks share one PSUM bank (128, 8, 64);
    reduce_max(negate=True) feeds exp's per-partition bias.
  * All Ln work is deferred to end-of-kernel activations over the
    (128, 98) gathered sums (avoids exp/ln ACT-table ping-pong).
  * Per-core partial sum is reduced over partitions with a tiny f32
    matmul against ones; the host adds the 8 scalars and analytically
    removes the zero-row padding contribution.
"""

import numpy as np
import ml_dtypes

N, K, O = 100000, 64, 1024
EPS = 1e-6
CORES = 8
NSHARD = N // CORES          # 12500
BLK = 128
NBLK = 98                    # ceil(12500 / 128)
NPAD = NBLK * BLK            # 12544
BPS = 14                     # blocks per superblock (even: pairs don't split)
SBS = NBLK // BPS            # 7 superblocks
PAD_ROWS = NPAD - NSHARD     # 44 zero rows per core
GRP = 8                      # blocks per PSUM group (one full PSUM bank)

_cache = {}


def _build_module(use_fp8):
    import concourse.bacc as bacc
    import concourse.tile as tile
    from concourse import mybir

    # Force all activations (Exp/Ln/Copy) onto the one ACT table set that
    # contains them all — otherwise every Exp<->Ln switch pays a ~1.3us
    # ACT_TABLE_LOAD. Other sets are blanked (positions kept so the
    # act_func_set_id -> act_info.json index mapping stays valid).
    if not getattr(bacc, "_act_tables_patched", False):
        _orig_get = bacc.get_activation_tables

        def _only_ln_exp(arch):
            tabs = _orig_get(arch)
            return {
                name: (fns if name == "natural_log_exp_and_others" else set())
                for name, fns in tabs.items()
            }

        bacc.get_activation_tables = _only_ln_exp
        bacc._act_tables_patched = True

    f32 = mybir.dt.float32
    bf16 = mybir.dt.bfloat16
    cdt = mybir.dt.float8e4 if use_fp8 else bf16
    AX = mybir.AxisListType.X
    AF = mybir.ActivationFunctionType
    ALU = mybir.AluOpType
    DR = mybir.MatmulPerfMode.DoubleRow

    nc = bacc.Bacc("TRN2", target_bir_lowering=False, debug=False,
                   num_devices=CORES)
    cnts = nc.dram_tensor("cnts", [SBS, 128, BPS * 8, BLK], cdt,
                          kind="ExternalInput").ap()
    hiw = nc.dram_tensor("hiw", [128, 8, K], cdt,
                         kind="ExternalInput").ap()
    priorb = nc.dram_tensor("priorb", [1, GRP, K], bf16,
                            kind="ExternalInput").ap()
    onesb = nc.dram_tensor("onesb", [1, 128], bf16,
                           kind="ExternalInput").ap()
    ones = nc.dram_tensor("ones", [128, 1], f32, kind="ExternalInput").ap()
    out = nc.dram_tensor("out", [1, 1], f32, kind="ExternalOutput").ap()

    with tile.TileContext(nc, num_cores=CORES) as tc:
        with (
            tc.tile_pool(name="const", bufs=1) as const,
            tc.tile_pool(name="cnt", bufs=SBS) as cnt_pool,
            tc.tile_pool(name="work", bufs=4) as work,
            tc.tile_pool(name="psum", bufs=7, space="PSUM") as psum_pool,
            tc.tile_pool(name="fpsum", bufs=1, space="PSUM") as fpsum_pool,
        ):
            # hi gates the first matmul: it goes FIRST on the sync HWDGE
            # queue (64 KB, ~0.2us). The other constants ride the gpsimd
            # SWDGE queue and are only needed later (prior matmul is last
            # in each group).
            hi_sb = const.tile([128, 8, K], cdt)
            nc.sync.dma_start(out=hi_sb[:], in_=hiw)
            priorb_sb = const.tile([1, GRP, K], bf16)
            nc.gpsimd.dma_start(out=priorb_sb[:], in_=priorb)
            onesb_sb = const.tile([1, 128], bf16)
            nc.gpsimd.dma_start(out=onesb_sb[:], in_=onesb)
            ones_sb = const.tile([128, 1], f32)
            nc.gpsimd.dma_start(out=ones_sb[:], in_=ones)
            mg_all = const.tile([128, NBLK], f32)
            sg_all = const.tile([128, NBLK], bf16)
            # touch Exp and Ln once (into a slice that is later fully
            # overwritten, so DCE keeps it) so both ACT table loads overlap
            # the DMA-bound head instead of landing in the kernel tail
            warm = const.tile([1, 1], f32)
            nc.vector.memset(warm[:], 1.0)
            nc.scalar.activation(mg_all[0:1, 0:1], warm[:], AF.Exp)
            nc.scalar.activation(mg_all[0:1, 0:1], warm[:], AF.Ln)

            # Issue ALL counts DMAs up front: the full shard fits in SBUF,
            # so the DMA engines never wait on compute. Fine-grained splits
            # at the head (compute starts early) and tail (short drain).
            # Descriptor generation (~0.7us per dma_start) is spread across
            # four engines' queues so the head is not serialized on sync.
            # All issues ride the sync engine: it is otherwise idle, and
            # issuing from scalar would queue descriptors behind the Exps,
            # starving the DMA engines mid-kernel.
            cnt_tiles = []
            for s in range(SBS):
                cnt = cnt_pool.tile([128, BPS * 8, BLK], cdt)
                if s == 0:
                    splits = [0, 1, 2, 4, 7, 10, BPS]
                else:
                    splits = [0, 4, 7, 11, BPS]
                for a, b in zip(splits, splits[1:]):
                    nc.sync.dma_start(out=cnt[:, a * 8:b * 8, :],
                                      in_=cnts[s, :, a * 8:b * 8, :])
                cnt_tiles.append(cnt)

            def block_ap(b):
                return cnt_tiles[b // BPS], (b % BPS) * 8

            # 8-block groups while the DMA paces the kernel; 4-block groups
            # for the last two superblocks so the end-of-kernel drain chain
            # (max -> exp path -> sum per group) is shorter
            groups = [(8 * q, 8) for q in range(10)]
            groups += [(80 + 4 * q, 4) for q in range(4)]
            groups.append((96, 2))
            # incremental logsumexp tail: ln/sub/sum as columns complete
            segs = [(0, 24), (24, 48), (48, 72), (72, 96), (96, NBLK)]
            ls = const.tile([128, NBLK], f32)
            t3 = const.tile([128, NBLK], f32)
            acc8 = const.tile([128, len(segs)], f32)
            seg_after = {b: i for i, (a, b) in enumerate(segs)}

            def emit_seg(i):
                a, b = segs[i]
                nc.scalar.activation(ls[:, a:b], sg_all[:, a:b], AF.Ln)
                nc.vector.tensor_sub(t3[:, a:b], ls[:, a:b], mg_all[:, a:b])
                nc.vector.reduce_sum(acc8[:, i:i + 1], t3[:, a:b], axis=AX)

            from concourse.bass import AP as BassAP

            def k_broadcast(ap_2d):
                # view a [128, gn] slice as [128, gn, K] with 0-stride K dim
                return BassAP(ap_2d.tensor, ap_2d.offset,
                              list(ap_2d.ap) + [[0, K]])

            for gi, (g0, gn) in enumerate(groups):
                B8 = psum_pool.tile([128, GRP, K], mybir.dt.float32)
                for j in range(gn):                 # block within group
                    tile_b, off = block_ap(g0 + j)
                    # plain fp8 matmuls: DoubleRow halves stream cycles but
                    # the PE power-cap halves the clock in exchange, while
                    # the (serialized, unhidden) ldweights double in time —
                    # measured net loss. Non-DR runs at full clock.
                    for c in range(8):
                        nc.tensor.matmul(
                            B8[:, j, :],
                            lhsT=tile_b[:, off + c, :],
                            rhs=hi_sb[:, c, :],
                            start=(j == 0 and c == 0),
                            stop=False,
                            skip_group_check=True,
                        )
                # prior lands last via a 1-row bf16 matmul so the group's
                # first counts matmul is not gated on the SWDGE constants
                nc.tensor.matmul(
                    B8[:, :gn, :],
                    lhsT=onesb_sb[:],
                    rhs=priorb_sb[:, :gn, :],
                    start=False, stop=True,
                    skip_group_check=True,
                )
                nc.vector.reduce_max(mg_all[:, g0:g0 + gn], B8[:, :gn, :],
                                     axis=AX, negate=True)
                e8 = work.tile([128, GRP, K], bf16, tag="e8")
                if gi % 2 == 0:
                    # scalar path: per-block Exp reads PSUM with the negated
                    # max as per-partition bias
                    for j in range(gn):
                        nc.scalar.activation(
                            e8[:, j, :], B8[:, j, :], AF.Exp,
                            bias=mg_all[:, g0 + j:g0 + j + 1], scale=1.0)
                else:
                    # DVE path: x - max via one broadcast add, then a single
                    # grouped Exp. Alternating paths balances the two
                    # engines' queues during the end-of-kernel drain.
                    t5 = work.tile([128, GRP, K], bf16, tag="t5")
                    nc.vector.tensor_add(t5[:, :gn, :], B8[:, :gn, :],
                                         k_broadcast(mg_all[:, g0:g0 + gn]))
                    nc.scalar.activation(e8[:, :gn, :], t5[:, :gn, :],
                                         AF.Exp)
                with nc.allow_low_precision("sum of <=64 max-normalized "
                                            "exps; bf16 keeps 2x DVE mode"):
                    nc.vector.reduce_sum(sg_all[:, g0:g0 + gn],
                                         e8[:, :gn, :], axis=AX)
                if g0 + gn in seg_after:
                    emit_seg(seg_after[g0 + gn])

            accp = const.tile([128, 1], f32)
            nc.vector.reduce_sum(accp[:], acc8[:], axis=AX)
            fin_ps = fpsum_pool.tile([1, 1], f32)
            nc.tensor.matmul(fin_ps[:], lhsT=accp[:], rhs=ones_sb[:],
                             start=True, stop=True)
            fin_sb = const.tile([1, 1], f32)
            nc.scalar.copy(fin_sb[:], fin_ps[:])
            nc.sync.dma_start(out=out, in_=fin_sb[:])

    nc.finalize()
    return nc


def _prep_inputs(counts, otu_dist, comm_dist, use_fp8):
    np_dt = ml_dtypes.float8_e4m3 if use_fp8 else ml_dtypes.bfloat16
    log_otu = np.log(otu_dist.astype(np.float32) + np.float32(EPS))
    hi = log_otu.astype(np_dt)
    # [p, c, k] = x[k, c*128 + p]
    hiw = np.ascontiguousarray(
        hi.reshape(K, 8, BLK).transpose(2, 1, 0))      # (128, 8, 64)

    prior_vec = np.log(comm_dist.astype(np.float32) + np.float32(EPS))
    prior_b16 = prior_vec.astype(ml_dtypes.bfloat16)
    priorb = np.ascontiguousarray(
        np.broadcast_to(prior_b16[None, None, :], (1, GRP, K)))
    onesb = np.ones((1, 128), ml_dtypes.bfloat16)
    ones = np.ones((128, 1), np.float32)

    counts_q = counts.astype(np_dt)
    shards = []
    for i in range(CORES):
        sh = counts_q[i * NSHARD:(i + 1) * NSHARD]
        shp = np.zeros((NPAD, O), np_dt)
        shp[:NSHARD] = sh
        # (s, b, j, c, p) -> (s, p, b, c, j)
        arr = shp.reshape(SBS, BPS, BLK, 8, BLK).transpose(0, 4, 1, 3, 2)
        shards.append(np.ascontiguousarray(arr).reshape(SBS, 128,
                                                        BPS * 8, BLK))

    in_maps = [
        {"cnts": shards[i], "hiw": hiw, "priorb": priorb, "onesb": onesb,
         "ones": ones}
        for i in range(CORES)
    ]
    # per-particle value contributed by each all-zero padding row:
    # raw = bf16(prior)  ->  summand = logsumexp(bf16(prior))
    pad_val = _np_logsumexp(prior_b16.astype(np.float64))
    return in_maps, pad_val


def _np_logsumexp(v):
    m = np.max(v)
    return m + np.log(np.sum(np.exp(v - m)))


def kernel(counts, otu_dist, comm_dist):
    from concourse.bass_utils import run_bass_kernel_spmd

    counts = np.asarray(counts)
    fp8 = ml_dtypes.float8_e4m3
    use_fp8 = bool(
        np.array_equal(counts.astype(fp8).astype(np.float32),
                       counts.astype(np.float32)))

    key = ("nc", use_fp8)
    if key not in _cache:
        _cache[key] = _build_module(use_fp8)
    nc = _cache[key]

    in_maps, pad_val = _prep_inputs(counts, np.asarray(otu_dist),
                                    np.asarray(comm_dist), use_fp8)
    res = run_bass_kernel_spmd(nc, in_maps, list(range(CORES)))
    total = sum(float(res.results[c]["out"][0, 0]) for c in range(CORES))
    total -= CORES * PAD_ROWS * pad_val
    return np.float32(total)
